# revision 22
# baseline (speedup 1.0000x reference)
"""Trainium2 Bass kernel for nn_CausalAttention (diff-attention with QK-norm,
RoPE, GQA, tanh soft-cap, causal softmax).

Sharding: 8 cores = (batch b in {0,1}) x (kv-group j in {0..3}).
Each core handles one batch element and the 4 query heads of one kv head.

Precision/perf strategy (fp32 matmuls cost 4 cyc/row on TRN2):
  - big GEMMs (projections, scores, O-proj) run as 3 bf16 matmuls on an
    exact hi/lo bf16 split of each operand (error ~= dropped lo*lo term,
    ~1.6e-5 relative) at 1 cyc/row each
  - attn_diff^T is built by two accumulated fp32 PE transpose-mode matmuls
    (2 cyc/row) of 1/r1-scaled e1 and (-lambda/r2)-scaled e2
  - AV and everything touching attention weights stays fp32
  - RMSNorm rsqrt runs as Newton iteration on DVE so ScalarE only ever
    needs the exp_and_others table (tanh+exp), avoiding table reloads
"""

import os
import sys

import numpy as np

if "/opt/trn_rl_repo" not in sys.path:
    sys.path.insert(0, "/opt/trn_rl_repo")

import ml_dtypes
import concourse.bass as bass
import concourse.mybir as mybir
import concourse.tile as tile
from concourse import bacc
from concourse.bass_utils import run_bass_kernel_spmd
from concourse.masks import make_identity

B, S, D = 2, 2048, 1024
H, KV, HD = 16, 4, 64
G = H // KV          # q heads per kv head (= heads per core)
CAP = 50.0
EPS = 1e-6
SCALE = 1.0 / 8.0    # 1/sqrt(HD)
P = 128
NSB = S // P         # 16 s-blocks
EQ = G * 2 * HD      # 512 q-projection cols per core
EK = 2 * HD          # 128 k-projection cols per core
EV = HD              # 64  v-projection cols per core
EQK = EQ + EK        # 640 cols needing norm+rope
EALL = EQ + EK + EV  # 704 projection cols per core
NG = EQK // HD       # 10 rmsnorm groups
KT = D // P          # 8 contraction tiles
MASK_FILL = -1.0e4   # exp(50 * -1e4) == 0 in fp32

F32 = mybir.dt.float32
BF16 = mybir.dt.bfloat16
MULT = mybir.AluOpType.mult
ADD = mybir.AluOpType.add
SUB = mybir.AluOpType.subtract


def _build_nc():
    nc = bacc.Bacc()
    x_d = nc.declare_dram_parameter("x", [S, D], F32, isOutput=False)
    whi_d = nc.declare_dram_parameter("w_hi", [D, EALL], BF16, isOutput=False)
    wlo_d = nc.declare_dram_parameter("w_lo", [D, EALL], BF16, isOutput=False)
    wohi_d = nc.declare_dram_parameter("wo_hi", [2 * P, D], BF16, isOutput=False)
    wolo_d = nc.declare_dram_parameter("wo_lo", [2 * P, D], BF16, isOutput=False)
    cos_d = nc.declare_dram_parameter("cos_d", [S, HD], F32, isOutput=False)
    sin_d = nc.declare_dram_parameter("sin_s", [S, HD], F32, isOutput=False)
    wn_d = nc.declare_dram_parameter("wnorm", [EQK], F32, isOutput=False)
    lam_d = nc.declare_dram_parameter("lam", [1], F32, isOutput=False)
    y_d = nc.declare_dram_parameter("y", [S, D], F32, isOutput=True)

    Tanh = mybir.ActivationFunctionType.Tanh
    Exp = mybir.ActivationFunctionType.Exp

    with tile.TileContext(nc) as tc:
        with (
            tc.tile_pool(name="singles", bufs=1) as singles,
            tc.tile_pool(name="persist", bufs=1) as persist,
            tc.tile_pool(name="work", bufs=2) as work,
            tc.tile_pool(name="tpool", bufs=3) as tpool,
            tc.tile_pool(name="atpool", bufs=2) as atpool,
            tc.tile_pool(name="small", bufs=10) as small,
            tc.tile_pool(name="psA", bufs=2, space="PSUM") as psA,
            tc.tile_pool(name="psB", bufs=2, space="PSUM") as psB,
            tc.tile_pool(name="psAT", bufs=2, space="PSUM") as psAT,
        ):
            # ---- one-time setup ----
            w_hi = singles.tile([P, KT, EALL], BF16)
            nc.sync.dma_start(w_hi, whi_d.rearrange("(t p) e -> p t e", p=P))
            w_lo = singles.tile([P, KT, EALL], BF16)
            nc.sync.dma_start(w_lo, wlo_d.rearrange("(t p) e -> p t e", p=P))
            wo_hi = singles.tile([P, 2, D], BF16)
            nc.sync.dma_start(wo_hi, wohi_d.rearrange("(t p) e -> p t e", p=P))
            wo_lo = singles.tile([P, 2, D], BF16)
            nc.sync.dma_start(wo_lo, wolo_d.rearrange("(t p) e -> p t e", p=P))
            cos_sb = singles.tile([P, NSB, HD], F32)
            nc.sync.dma_start(cos_sb, cos_d.rearrange("(n p) f -> p n f", p=P))
            sin_sb = singles.tile([P, NSB, HD], F32)
            nc.sync.dma_start(sin_sb, sin_d.rearrange("(n p) f -> p n f", p=P))

            def part_bcast(handle):
                ap = handle[:]
                return bass.AP(tensor=ap.tensor, offset=ap.offset, ap=[[0, P], *ap.ap])

            wn_sb = singles.tile([P, EQK], F32)
            nc.gpsimd.dma_start(wn_sb, part_bcast(wn_d))
            lam_sb = singles.tile([P, 1], F32)
            nc.gpsimd.dma_start(lam_sb, part_bcast(lam_d))
            ident = singles.tile([P, P], F32)
            make_identity(nc, ident)

            # persistent per-core activation storage
            v_sb = [persist.tile([P, EV], F32, name=f"v{i}", tag=f"v{i}")
                    for i in range(NSB)]
            qThi = [persist.tile([P, G, P], BF16, name=f"qThi{i}", tag=f"qThi{i}")
                    for i in range(NSB)]
            qTlo = [persist.tile([P, G, P], BF16, name=f"qTlo{i}", tag=f"qTlo{i}")
                    for i in range(NSB)]
            kThi = [persist.tile([P, 512], BF16, name=f"kThi{i}", tag=f"kThi{i}")
                    for i in range(NSB // 4)]
            kTlo = [persist.tile([P, 512], BF16, name=f"kTlo{i}", tag=f"kTlo{i}")
                    for i in range(NSB // 4)]

            def bcast_groups(src2d, n):
                return bass.AP(
                    tensor=src2d.tensor,
                    offset=src2d.offset,
                    ap=[src2d.ap[0], [0, n], src2d.ap[-1]],
                )

            def hilo_evict(psrc, hi, lo):
                """psum fp32 -> bf16 hi + bf16 lo (exact split)"""
                nc.vector.tensor_copy(hi, psrc)
                nc.vector.scalar_tensor_tensor(
                    out=lo, in0=hi, scalar=-1.0, in1=psrc, op0=MULT, op1=ADD
                )

            def phase1(si):
                """projections + rmsnorm + rope + transposes for s-block si"""
                x_sb = work.tile([P, D], F32, tag="x")
                nc.sync.dma_start(x_sb, x_d[si * P:(si + 1) * P, :])
                # transpose x block: [s,d] -> [d,s], split to bf16 hi/lo
                xThi = work.tile([P, KT, P], BF16, tag="xThi")
                xTlo = work.tile([P, KT, P], BF16, tag="xTlo")
                for half in range(2):
                    pt = psB.tile([P, 512], F32, tag="pose")
                    for t in range(4):
                        tt = 4 * half + t
                        nc.tensor.transpose(
                            pt[:, t * P:(t + 1) * P], x_sb[:, tt * P:(tt + 1) * P],
                            ident,
                        )
                    sl = slice(4 * half, 4 * half + 4)
                    hilo_evict(pt, xThi[:, sl, :], xTlo[:, sl, :])
                # projections via hi/lo bf16 3-matmul
                pp = psA.tile([P, 1024], F32, tag="A")
                for t in range(KT):
                    first, last = t == 0, t == KT - 1
                    for co, cw in ((0, 512), (512, 192)):
                        po = pp[:, co:co + cw]
                        wsl = slice(co, co + cw)
                        nc.tensor.matmul(
                            po, xThi[:, t, :], w_hi[:, t, wsl],
                            start=first, stop=False,
                        )
                        nc.tensor.matmul(
                            po, xThi[:, t, :], w_lo[:, t, wsl],
                            start=False, stop=False,
                        )
                        nc.tensor.matmul(
                            po, xTlo[:, t, :], w_hi[:, t, wsl],
                            start=False, stop=last,
                        )
                # v: plain eviction (fp32)
                nc.vector.tensor_copy(v_sb[si], pp[:, EQK:EALL])
                # rmsnorm stats
                qk0 = work.tile([P, EQK], F32, tag="qk0")
                nc.vector.tensor_copy(qk0, pp[:, 0:EQK])
                sq = work.tile([P, EQK], F32, tag="m1")
                nc.gpsimd.tensor_mul(sq, qk0, qk0)
                ssq = small.tile([P, NG], F32, tag="ssq")
                nc.vector.tensor_reduce(
                    ssq, sq.rearrange("p (g d) -> p g d", d=HD),
                    axis=mybir.AxisListType.X, op=ADD,
                )
                # a = mean_sq + eps; rinv = rsqrt(a) via Newton on DVE
                # (keeps ScalarE on the exp/tanh table only)
                aa = small.tile([P, NG], F32, tag="aa")
                nc.vector.tensor_scalar(
                    out=aa, in0=ssq, scalar1=1.0 / HD, scalar2=EPS,
                    op0=MULT, op1=ADD,
                )
                rinv = small.tile([P, NG], F32, tag="rinv")
                nc.vector.reciprocal(rinv, aa)
                nc.vector.tensor_scalar_min(rinv, rinv, 1.0)
                t_n = small.tile([P, NG], F32, tag="t_n")
                for _ in range(5):
                    nc.vector.tensor_mul(t_n, rinv, rinv)
                    nc.vector.tensor_mul(t_n, t_n, aa)
                    nc.vector.tensor_scalar(
                        out=t_n, in0=t_n, scalar1=-0.5, scalar2=1.5,
                        op0=MULT, op1=ADD,
                    )
                    nc.vector.tensor_mul(rinv, rinv, t_n)
                # apply 1/rms and norm weight
                qk = work.tile([P, EQK], F32, tag="qk")
                for g in range(NG):
                    sl = slice(g * HD, (g + 1) * HD)
                    nc.vector.scalar_tensor_tensor(
                        out=qk[:, sl], in0=qk0[:, sl], scalar=rinv[:, g:g + 1],
                        in1=wn_sb[:, sl], op0=MULT, op1=MULT,
                    )
                # rope: out = qk * cos_dup + swap(qk) * sin_sign
                qkv = qk.rearrange("p (n two) -> p n two", two=2)
                xr = work.tile([P, EQK], F32, tag="qk0")
                xrv = xr.rearrange("p (n two) -> p n two", two=2)
                nc.gpsimd.tensor_copy(xrv[:, :, 0:1], qkv[:, :, 1:2])
                nc.gpsimd.tensor_copy(xrv[:, :, 1:2], qkv[:, :, 0:1])
                cosb = bcast_groups(cos_sb[:, si, :], NG)
                sinb = bcast_groups(sin_sb[:, si, :], NG)
                m1 = work.tile([P, EQK], F32, tag="m1")
                nc.gpsimd.tensor_mul(m1, qk, cosb)
                nc.vector.tensor_mul(xr, xr, sinb)
                nc.vector.tensor_add(qk, m1, xr)
                # transpose q heads and k to [dim, s]; split bf16 hi/lo
                pq = psB.tile([P, 512], F32, tag="pose")
                for h in range(G):
                    nc.tensor.transpose(
                        pq[:, h * P:(h + 1) * P], qk[:, h * P:(h + 1) * P], ident
                    )
                hilo_evict(pq, qThi[si], qTlo[si])
                pk = psB.tile([P, 512], F32, tag="pose")
                nc.tensor.transpose(pk[:, 0:P], qk[:, EQ:EQK], ident)
                ksl = slice((si % 4) * P, (si % 4 + 1) * P)
                hilo_evict(pk[:, 0:P], kThi[si // 4][:, ksl], kTlo[si // 4][:, ksl])

            def attention(qb):
                """attention + O-projection for q-block qb (all 4 heads)"""
                nkb = qb + 1
                L = nkb * P
                oThi = [small.tile([P, P], BF16, name=f"oThi{qb}_{hp}", tag=f"oThi{hp}")
                        for hp in range(2)]
                oTlo = [small.tile([P, P], BF16, name=f"oTlo{qb}_{hp}", tag=f"oTlo{hp}")
                        for hp in range(2)]
                for hp in range(2):
                    at_sb = []
                    for hh in range(2):
                        h = 2 * hp + hh
                        t12 = tpool.tile([P, 2, S], F32, tag="t")
                        t1 = t12[:, 0, :]
                        t2 = t12[:, 1, :]
                        # scores: hi/lo 3-matmul, s1/s2 row-paired
                        for kc in range(0, L, 512):
                            w = min(512, L - kc)
                            sc = psA.tile([P, 1024], F32, tag="A")
                            ci = kc // 512
                            for off, qh in ((0, 0), (512, HD)):
                                khi = kThi[ci][qh:qh + HD, 0:w]
                                klo = kTlo[ci][qh:qh + HD, 0:w]
                                qhi = qThi[qb][qh:qh + HD, h, :]
                                qlo = qTlo[qb][qh:qh + HD, h, :]
                                po = sc[:, off:off + w]
                                nc.tensor.matmul(po, qhi, khi, start=True, stop=False)
                                nc.tensor.matmul(po, qhi, klo, start=False, stop=False)
                                nc.tensor.matmul(po, qlo, khi, start=False, stop=True)
                            # one tanh over both score halves
                            nc.scalar.activation(
                                t12[:, :, kc:kc + w],
                                sc.rearrange("p (m c) -> p m c", m=2)[:, :, 0:w],
                                Tanh, scale=SCALE / CAP,
                            )
                        # causal mask on diagonal block (keep where row >= col)
                        for t in (t1, t2):
                            nc.gpsimd.affine_select(
                                out=t[:, qb * P:L], in_=t[:, qb * P:L],
                                compare_op=mybir.AluOpType.is_ge, fill=MASK_FILL,
                                base=0, pattern=[[-1, P]], channel_multiplier=1,
                            )
                        # exp in place with row-sum accumulation
                        r1 = small.tile([P, 1], F32, tag="r")
                        r2 = small.tile([P, 1], F32, tag="r")
                        nc.scalar.activation(
                            t1[:, 0:L], t1[:, 0:L], Exp, scale=CAP, accum_out=r1
                        )
                        nc.scalar.activation(
                            t2[:, 0:L], t2[:, 0:L], Exp, scale=CAP, accum_out=r2
                        )
                        r1i = small.tile([P, 1], F32, tag="r")
                        nc.vector.reciprocal(r1i, r1)
                        r2i = small.tile([P, 1], F32, tag="r")
                        nc.vector.reciprocal(r2i, r2)
                        nr2i = small.tile([P, 1], F32, tag="r")
                        nc.vector.tensor_scalar(
                            out=nr2i, in0=r2i, scalar1=lam_sb[:, 0:1], scalar2=-1.0,
                            op0=MULT, op1=MULT,
                        )
                        # normalize in place: e1 *= 1/r1 (DVE), e2 *= -lam/r2
                        nc.vector.tensor_scalar_mul(t1[:, 0:L], t1[:, 0:L],
                                                    r1i[:, 0:1])
                        nc.vector.tensor_scalar_mul(t2[:, 0:L], t2[:, 0:L],
                                                    nr2i[:, 0:1])
                        # attn_diff^T via two accumulated fp32 transposes
                        a_sb = atpool.tile([P, S], F32, tag="at")
                        for kc in range(0, L, 512):
                            w = min(512, L - kc)
                            at4 = psAT.tile([P, 512], F32, tag="atpo")
                            # one bank-clearing start, then per-element
                            # overwrite (e1, has_written unset) / accumulate
                            # (e2, over e1's bits)
                            for kk in range(0, w, P):
                                sl = slice(kc + kk, kc + kk + P)
                                nc.tensor.matmul(
                                    at4[:, kk:kk + P], t1[:, sl], ident,
                                    is_transpose=True, start=(kk == 0), stop=False,
                                )
                            for kk in range(0, w, P):
                                sl = slice(kc + kk, kc + kk + P)
                                nc.tensor.matmul(
                                    at4[:, kk:kk + P], t2[:, sl], ident,
                                    is_transpose=True, start=False,
                                    stop=(kk + P >= w),
                                )
                            nc.vector.tensor_copy(a_sb[:, kc:kc + w], at4[:, 0:w])
                        at_sb.append(a_sb)
                    # AV: head pair via column tiling
                    po = psAT.tile([P, P], F32, tag="atpo", padded_shape=[P, 512])
                    for kb in range(nkb):
                        sl = slice(kb * P, (kb + 1) * P)
                        nc.tensor.matmul(
                            po[0:HD, :], v_sb[kb], at_sb[0][:, sl],
                            start=(kb == 0), stop=(kb == nkb - 1),
                            tile_position=(0, 0),
                        )
                        nc.tensor.matmul(
                            po[HD:P, :], v_sb[kb], at_sb[1][:, sl],
                            start=(kb == 0), stop=(kb == nkb - 1),
                            tile_position=(0, 64),
                        )
                    hilo_evict(po, oThi[hp], oTlo[hp])
                # O-projection (hi/lo bf16 3-matmul)
                y_sb = work.tile([P, D], F32, tag="y")
                for ch in range(2):
                    py = psB.tile([P, 512], F32, tag="pose")
                    sl = slice(ch * 512, (ch + 1) * 512)
                    nc.tensor.matmul(py, oThi[0], wo_hi[:, 0, sl],
                                     start=True, stop=False)
                    nc.tensor.matmul(py, oThi[0], wo_lo[:, 0, sl],
                                     start=False, stop=False)
                    nc.tensor.matmul(py, oTlo[0], wo_hi[:, 0, sl],
                                     start=False, stop=False)
                    nc.tensor.matmul(py, oThi[1], wo_hi[:, 1, sl],
                                     start=False, stop=False)
                    nc.tensor.matmul(py, oThi[1], wo_lo[:, 1, sl],
                                     start=False, stop=False)
                    nc.tensor.matmul(py, oTlo[1], wo_hi[:, 1, sl],
                                     start=False, stop=True)
                    nc.vector.tensor_copy(y_sb[:, sl], py)
                nc.sync.dma_start(y_d[qb * P:(qb + 1) * P, :], y_sb)

            # software pipeline: keep phase1 two s-blocks ahead so the PE
            # always has attention matmuls available while DVE/GPSIMD run
            # the norm/rope chain of upcoming blocks
            import os as _os
            LOOKAHEAD = int(_os.environ.get("K_LOOKAHEAD", "2"))
            for si in range(min(LOOKAHEAD, NSB)):
                phase1(si)
            for si in range(NSB):
                attention(si)
                if si + LOOKAHEAD < NSB:
                    phase1(si + LOOKAHEAD)

    nc.finalize()
    return nc


_NC = None


def _get_nc():
    global _NC
    if _NC is None:
        _NC = _build_nc()
    return _NC


def _hilo(a):
    hi = a.astype(ml_dtypes.bfloat16)
    lo = (a - hi.astype(np.float32)).astype(ml_dtypes.bfloat16)
    return hi, lo


def kernel(x, rope_freqs, wq, wk, wv, wo, q_norm_w, k_norm_w, diff_lambda):
    x = np.asarray(x, dtype=np.float32)
    rope_freqs = np.asarray(rope_freqs, dtype=np.float32)
    wq, wk, wv, wo = (np.asarray(a, dtype=np.float32) for a in (wq, wk, wv, wo))
    q_norm_w = np.asarray(q_norm_w, dtype=np.float32)
    k_norm_w = np.asarray(k_norm_w, dtype=np.float32)
    diff_lambda = np.asarray(diff_lambda, dtype=np.float32)

    cos = np.repeat(rope_freqs[:, :, 0], 2, axis=1).astype(np.float32)
    sin = np.repeat(rope_freqs[:, :, 1], 2, axis=1).astype(np.float32)
    sin_s = sin.copy()
    sin_s[:, 0::2] *= -1.0
    wnorm = np.concatenate(
        [np.tile(q_norm_w, 2 * G), np.tile(k_norm_w, 2)]
    ).astype(np.float32)

    in_maps = []
    for c in range(8):
        b, j = divmod(c, KV)
        w_all_t = np.ascontiguousarray(
            np.concatenate(
                [
                    wq[EQ * j:EQ * (j + 1), :],
                    wk[EK * j:EK * (j + 1), :],
                    wv[EV * j:EV * (j + 1), :],
                ],
                axis=0,
            ).T
        )
        wo_t = np.ascontiguousarray(wo[:, 2 * P * j:2 * P * (j + 1)].T)
        w_hi, w_lo = _hilo(w_all_t)
        wo_hi, wo_lo = _hilo(wo_t)
        in_maps.append(
            {
                "x": np.ascontiguousarray(x[b]),
                "w_hi": w_hi,
                "w_lo": w_lo,
                "wo_hi": wo_hi,
                "wo_lo": wo_lo,
                "cos_d": cos,
                "sin_s": sin_s,
                "wnorm": wnorm,
                "lam": diff_lambda.reshape(1),
            }
        )

    nc = _get_nc()
    trace = os.environ.get("KERNEL_TRACE") == "1"
    res = run_bass_kernel_spmd(nc, in_maps, core_ids=list(range(8)), trace=trace)
    if trace and res.exec_time_ns is not None:
        print(f"HW exec time: {res.exec_time_ns} ns")

    out = np.zeros((B, S, D), dtype=np.float32)
    for c in range(8):
        b = c // KV
        out[b] += res.results[c]["y"]
    return out


# revision 27
# speedup vs baseline: 1.0157x; 1.0157x over previous
"""Trainium2 Bass kernel for nn_CausalAttention (diff-attention with QK-norm,
RoPE, GQA, tanh soft-cap, causal softmax).

Sharding: 8 cores = (batch b in {0,1}) x (kv-group j in {0..3}).
Each core handles one batch element and the 4 query heads of one kv head.

Precision/perf strategy (fp32 matmuls cost 4 cyc/row on TRN2):
  - big GEMMs (projections, scores, O-proj) run as 3 bf16 matmuls on an
    exact hi/lo bf16 split of each operand (error ~= dropped lo*lo term,
    ~1.6e-5 relative) at 1 cyc/row each
  - attn_diff^T is built by two accumulated fp32 PE transpose-mode matmuls
    (2 cyc/row) of 1/r1-scaled e1 and (-lambda/r2)-scaled e2
  - AV and everything touching attention weights stays fp32
  - RMSNorm rsqrt runs as Newton iteration on DVE so ScalarE only ever
    needs the exp_and_others table (tanh+exp), avoiding table reloads
"""

import os
import sys

import numpy as np

if "/opt/trn_rl_repo" not in sys.path:
    sys.path.insert(0, "/opt/trn_rl_repo")

import ml_dtypes
import concourse.bass as bass
import concourse.mybir as mybir
import concourse.tile as tile
from concourse import bacc
from concourse.bass_utils import run_bass_kernel_spmd
from concourse.masks import make_identity

B, S, D = 2, 2048, 1024
H, KV, HD = 16, 4, 64
G = H // KV          # q heads per kv head (= heads per core)
CAP = 50.0
EPS = 1e-6
SCALE = 1.0 / 8.0    # 1/sqrt(HD)
P = 128
NSB = S // P         # 16 s-blocks
EQ = G * 2 * HD      # 512 q-projection cols per core
EK = 2 * HD          # 128 k-projection cols per core
EV = HD              # 64  v-projection cols per core
EQK = EQ + EK        # 640 cols needing norm+rope
EALL = EQ + EK + EV  # 704 projection cols per core
NG = EQK // HD       # 10 rmsnorm groups
KT = D // P          # 8 contraction tiles
MASK_FILL = -1.0e4   # exp(50 * -1e4) == 0 in fp32

F32 = mybir.dt.float32
BF16 = mybir.dt.bfloat16
MULT = mybir.AluOpType.mult
ADD = mybir.AluOpType.add
SUB = mybir.AluOpType.subtract


def _build_nc():
    nc = bacc.Bacc()
    x_d = nc.declare_dram_parameter("x", [S, D], F32, isOutput=False)
    whi_d = nc.declare_dram_parameter("w_hi", [D, EALL], BF16, isOutput=False)
    wlo_d = nc.declare_dram_parameter("w_lo", [D, EALL], BF16, isOutput=False)
    wohi_d = nc.declare_dram_parameter("wo_hi", [2 * P, D], BF16, isOutput=False)
    wolo_d = nc.declare_dram_parameter("wo_lo", [2 * P, D], BF16, isOutput=False)
    cos_d = nc.declare_dram_parameter("cos_d", [S, HD], F32, isOutput=False)
    sin_d = nc.declare_dram_parameter("sin_s", [S, HD], F32, isOutput=False)
    wn_d = nc.declare_dram_parameter("wnorm", [EQK], F32, isOutput=False)
    lam_d = nc.declare_dram_parameter("lam", [1], F32, isOutput=False)
    y_d = nc.declare_dram_parameter("y", [S, D], F32, isOutput=True)

    Tanh = mybir.ActivationFunctionType.Tanh
    Exp = mybir.ActivationFunctionType.Exp

    with tile.TileContext(nc) as tc:
        with (
            tc.tile_pool(name="singles", bufs=1) as singles,
            tc.tile_pool(name="persist", bufs=1) as persist,
            tc.tile_pool(name="work", bufs=2) as work,
            tc.tile_pool(name="tpool", bufs=3) as tpool,
            tc.tile_pool(name="atpool", bufs=3) as atpool,
            tc.tile_pool(name="small", bufs=9) as small,
            tc.tile_pool(name="psA", bufs=2, space="PSUM") as psA,
            tc.tile_pool(name="psB", bufs=2, space="PSUM") as psB,
            tc.tile_pool(name="psAT", bufs=2, space="PSUM") as psAT,
        ):
            # ---- one-time setup ----
            w_hi = singles.tile([P, KT, EALL], BF16)
            nc.sync.dma_start(w_hi, whi_d.rearrange("(t p) e -> p t e", p=P))
            w_lo = singles.tile([P, KT, EALL], BF16)
            nc.sync.dma_start(w_lo, wlo_d.rearrange("(t p) e -> p t e", p=P))
            wo_hi = singles.tile([P, 2, D], BF16)
            nc.sync.dma_start(wo_hi, wohi_d.rearrange("(t p) e -> p t e", p=P))
            wo_lo = singles.tile([P, 2, D], BF16)
            nc.sync.dma_start(wo_lo, wolo_d.rearrange("(t p) e -> p t e", p=P))
            cos_sb = singles.tile([P, NSB, HD], F32)
            nc.sync.dma_start(cos_sb, cos_d.rearrange("(n p) f -> p n f", p=P))
            sin_sb = singles.tile([P, NSB, HD], F32)
            nc.sync.dma_start(sin_sb, sin_d.rearrange("(n p) f -> p n f", p=P))

            def part_bcast(handle):
                ap = handle[:]
                return bass.AP(tensor=ap.tensor, offset=ap.offset, ap=[[0, P], *ap.ap])

            wn_sb = singles.tile([P, EQK], F32)
            nc.gpsimd.dma_start(wn_sb, part_bcast(wn_d))
            lam_sb = singles.tile([P, 1], F32)
            nc.gpsimd.dma_start(lam_sb, part_bcast(lam_d))
            ident = singles.tile([P, P], F32)
            make_identity(nc, ident)

            # persistent per-core activation storage
            v_sb = [persist.tile([P, EV], F32, name=f"v{i}", tag=f"v{i}")
                    for i in range(NSB)]
            qThi = [persist.tile([P, G, P], BF16, name=f"qThi{i}", tag=f"qThi{i}")
                    for i in range(NSB)]
            qTlo = [persist.tile([P, G, P], BF16, name=f"qTlo{i}", tag=f"qTlo{i}")
                    for i in range(NSB)]
            kThi = [persist.tile([P, 512], BF16, name=f"kThi{i}", tag=f"kThi{i}")
                    for i in range(NSB // 4)]
            kTlo = [persist.tile([P, 512], BF16, name=f"kTlo{i}", tag=f"kTlo{i}")
                    for i in range(NSB // 4)]

            def bcast_groups(src2d, n):
                return bass.AP(
                    tensor=src2d.tensor,
                    offset=src2d.offset,
                    ap=[src2d.ap[0], [0, n], src2d.ap[-1]],
                )

            def hilo_evict(psrc, hi, lo):
                """psum fp32 -> bf16 hi + bf16 lo (exact split)"""
                nc.vector.tensor_copy(hi, psrc)
                nc.vector.scalar_tensor_tensor(
                    out=lo, in0=hi, scalar=-1.0, in1=psrc, op0=MULT, op1=ADD
                )

            def phase1(si):
                """projections + rmsnorm + rope + transposes for s-block si"""
                x_sb = work.tile([P, D], F32, tag="x")
                nc.scalar.dma_start(x_sb, x_d[si * P:(si + 1) * P, :])
                # transpose x block: [s,d] -> [d,s], split to bf16 hi/lo
                xThi = work.tile([P, KT, P], BF16, tag="xThi")
                xTlo = work.tile([P, KT, P], BF16, tag="xTlo")
                for half in range(2):
                    pt = psB.tile([P, 512], F32, tag="pose")
                    for t in range(4):
                        tt = 4 * half + t
                        nc.tensor.transpose(
                            pt[:, t * P:(t + 1) * P], x_sb[:, tt * P:(tt + 1) * P],
                            ident,
                        )
                    sl = slice(4 * half, 4 * half + 4)
                    hilo_evict(pt, xThi[:, sl, :], xTlo[:, sl, :])
                # projections via hi/lo bf16 3-matmul
                pp = psA.tile([P, 1024], F32, tag="A")
                for t in range(KT):
                    first, last = t == 0, t == KT - 1
                    for co, cw in ((0, 512), (512, 192)):
                        po = pp[:, co:co + cw]
                        wsl = slice(co, co + cw)
                        nc.tensor.matmul(
                            po, xThi[:, t, :], w_hi[:, t, wsl],
                            start=first, stop=False,
                        )
                        nc.tensor.matmul(
                            po, xThi[:, t, :], w_lo[:, t, wsl],
                            start=False, stop=False,
                        )
                        nc.tensor.matmul(
                            po, xTlo[:, t, :], w_hi[:, t, wsl],
                            start=False, stop=last,
                        )
                # v: plain eviction (fp32)
                nc.vector.tensor_copy(v_sb[si], pp[:, EQK:EALL])
                # rmsnorm stats
                qk0 = work.tile([P, EQK], F32, tag="qk0")
                nc.vector.tensor_copy(qk0, pp[:, 0:EQK])
                sq = work.tile([P, EQK], F32, tag="m1")
                nc.gpsimd.tensor_mul(sq, qk0, qk0)
                ssq = small.tile([P, NG], F32, tag="ssq")
                nc.vector.tensor_reduce(
                    ssq, sq.rearrange("p (g d) -> p g d", d=HD),
                    axis=mybir.AxisListType.X, op=ADD,
                )
                # a = mean_sq + eps; rinv = rsqrt(a) via Newton on DVE
                # (keeps ScalarE on the exp/tanh table only)
                aa = small.tile([P, NG], F32, tag="aa")
                nc.vector.tensor_scalar(
                    out=aa, in0=ssq, scalar1=1.0 / HD, scalar2=EPS,
                    op0=MULT, op1=ADD,
                )
                rinv = small.tile([P, NG], F32, tag="rinv")
                nc.vector.reciprocal(rinv, aa)
                nc.vector.tensor_scalar_min(rinv, rinv, 1.0)
                t_n = small.tile([P, NG], F32, tag="t_n")
                for _ in range(5):
                    nc.vector.tensor_mul(t_n, rinv, rinv)
                    nc.vector.tensor_mul(t_n, t_n, aa)
                    nc.vector.tensor_scalar(
                        out=t_n, in0=t_n, scalar1=-0.5, scalar2=1.5,
                        op0=MULT, op1=ADD,
                    )
                    nc.vector.tensor_mul(rinv, rinv, t_n)
                # apply 1/rms and norm weight
                qk = work.tile([P, EQK], F32, tag="qk")
                for g in range(NG):
                    sl = slice(g * HD, (g + 1) * HD)
                    nc.vector.scalar_tensor_tensor(
                        out=qk[:, sl], in0=qk0[:, sl], scalar=rinv[:, g:g + 1],
                        in1=wn_sb[:, sl], op0=MULT, op1=MULT,
                    )
                # rope: out = qk * cos_dup + swap(qk) * sin_sign
                qkv = qk.rearrange("p (n two) -> p n two", two=2)
                xr = work.tile([P, EQK], F32, tag="qk0")
                xrv = xr.rearrange("p (n two) -> p n two", two=2)
                nc.gpsimd.tensor_copy(xrv[:, :, 0:1], qkv[:, :, 1:2])
                nc.gpsimd.tensor_copy(xrv[:, :, 1:2], qkv[:, :, 0:1])
                cosb = bcast_groups(cos_sb[:, si, :], NG)
                sinb = bcast_groups(sin_sb[:, si, :], NG)
                m1 = work.tile([P, EQK], F32, tag="m1")
                nc.gpsimd.tensor_mul(m1, qk, cosb)
                nc.vector.tensor_mul(xr, xr, sinb)
                nc.vector.tensor_add(qk, m1, xr)
                # transpose q heads and k to [dim, s]; split bf16 hi/lo
                pq = psB.tile([P, 512], F32, tag="pose")
                for h in range(G):
                    nc.tensor.transpose(
                        pq[:, h * P:(h + 1) * P], qk[:, h * P:(h + 1) * P], ident
                    )
                hilo_evict(pq, qThi[si], qTlo[si])
                pk = psB.tile([P, 512], F32, tag="pose")
                nc.tensor.transpose(pk[:, 0:P], qk[:, EQ:EQK], ident)
                ksl = slice((si % 4) * P, (si % 4 + 1) * P)
                hilo_evict(pk[:, 0:P], kThi[si // 4][:, ksl], kTlo[si // 4][:, ksl])

            def attention(qb):
                """attention + O-projection for q-block qb (all 4 heads)"""
                nkb = qb + 1
                L = nkb * P
                oThi = [small.tile([P, P], BF16, name=f"oThi{qb}_{hp}", tag=f"oThi{hp}")
                        for hp in range(2)]
                oTlo = [small.tile([P, P], BF16, name=f"oTlo{qb}_{hp}", tag=f"oTlo{hp}")
                        for hp in range(2)]
                for hp in range(2):
                    at_sb = []
                    for hh in range(2):
                        h = 2 * hp + hh
                        t12 = tpool.tile([P, 2, S], F32, tag="t")
                        t1 = t12[:, 0, :]
                        t2 = t12[:, 1, :]
                        # scores: hi/lo 3-matmul, s1/s2 row-paired
                        for kc in range(0, L, 512):
                            w = min(512, L - kc)
                            sc = psA.tile([P, 1024], F32, tag="A")
                            ci = kc // 512
                            for off, qh in ((0, 0), (512, HD)):
                                khi = kThi[ci][qh:qh + HD, 0:w]
                                klo = kTlo[ci][qh:qh + HD, 0:w]
                                qhi = qThi[qb][qh:qh + HD, h, :]
                                qlo = qTlo[qb][qh:qh + HD, h, :]
                                po = sc[:, off:off + w]
                                nc.tensor.matmul(po, qhi, khi, start=True, stop=False)
                                nc.tensor.matmul(po, qhi, klo, start=False, stop=False)
                                nc.tensor.matmul(po, qlo, khi, start=False, stop=True)
                            # one tanh over both score halves
                            nc.scalar.activation(
                                t12[:, :, kc:kc + w],
                                sc.rearrange("p (m c) -> p m c", m=2)[:, :, 0:w],
                                Tanh, scale=SCALE / CAP,
                            )
                        # causal mask on diagonal block (keep where row >= col)
                        for t in (t1, t2):
                            nc.gpsimd.affine_select(
                                out=t[:, qb * P:L], in_=t[:, qb * P:L],
                                compare_op=mybir.AluOpType.is_ge, fill=MASK_FILL,
                                base=0, pattern=[[-1, P]], channel_multiplier=1,
                            )
                        # exp in place with row-sum accumulation
                        r1 = small.tile([P, 1], F32, tag="r")
                        r2 = small.tile([P, 1], F32, tag="r")
                        nc.scalar.activation(
                            t1[:, 0:L], t1[:, 0:L], Exp, scale=CAP, accum_out=r1
                        )
                        nc.scalar.activation(
                            t2[:, 0:L], t2[:, 0:L], Exp, scale=CAP, accum_out=r2
                        )
                        r1i = small.tile([P, 1], F32, tag="r")
                        nc.vector.reciprocal(r1i, r1)
                        r2i = small.tile([P, 1], F32, tag="r")
                        nc.vector.reciprocal(r2i, r2)
                        nr2i = small.tile([P, 1], F32, tag="r")
                        nc.vector.tensor_scalar(
                            out=nr2i, in0=r2i, scalar1=lam_sb[:, 0:1], scalar2=-1.0,
                            op0=MULT, op1=MULT,
                        )
                        # normalize in place: e1 *= 1/r1 (DVE), e2 *= -lam/r2
                        nc.vector.tensor_scalar_mul(t1[:, 0:L], t1[:, 0:L],
                                                    r1i[:, 0:1])
                        nc.vector.tensor_scalar_mul(t2[:, 0:L], t2[:, 0:L],
                                                    nr2i[:, 0:1])
                        # attn_diff^T via two accumulated fp32 transposes
                        a_sb = atpool.tile([P, S], F32, tag="at")
                        for kc in range(0, L, 512):
                            w = min(512, L - kc)
                            at4 = psAT.tile([P, 512], F32, tag="atpo")
                            # one bank-clearing start, then per-element
                            # overwrite (e1, has_written unset) / accumulate
                            # (e2, over e1's bits)
                            for kk in range(0, w, P):
                                sl = slice(kc + kk, kc + kk + P)
                                nc.tensor.matmul(
                                    at4[:, kk:kk + P], t1[:, sl], ident,
                                    is_transpose=True, start=(kk == 0), stop=False,
                                )
                            for kk in range(0, w, P):
                                sl = slice(kc + kk, kc + kk + P)
                                nc.tensor.matmul(
                                    at4[:, kk:kk + P], t2[:, sl], ident,
                                    is_transpose=True, start=False,
                                    stop=(kk + P >= w),
                                )
                            nc.vector.tensor_copy(a_sb[:, kc:kc + w], at4[:, 0:w])
                        at_sb.append(a_sb)
                    # AV: head pair via column tiling
                    po = psAT.tile([P, P], F32, tag="atpo", padded_shape=[P, 512])
                    for kb in range(nkb):
                        sl = slice(kb * P, (kb + 1) * P)
                        nc.tensor.matmul(
                            po[0:HD, :], v_sb[kb], at_sb[0][:, sl],
                            start=(kb == 0), stop=(kb == nkb - 1),
                            tile_position=(0, 0),
                        )
                        nc.tensor.matmul(
                            po[HD:P, :], v_sb[kb], at_sb[1][:, sl],
                            start=(kb == 0), stop=(kb == nkb - 1),
                            tile_position=(0, 64),
                        )
                    hilo_evict(po, oThi[hp], oTlo[hp])
                # O-projection (hi/lo bf16 3-matmul)
                y_sb = work.tile([P, D], F32, tag="y")
                for ch in range(2):
                    py = psB.tile([P, 512], F32, tag="pose")
                    sl = slice(ch * 512, (ch + 1) * 512)
                    nc.tensor.matmul(py, oThi[0], wo_hi[:, 0, sl],
                                     start=True, stop=False)
                    nc.tensor.matmul(py, oThi[0], wo_lo[:, 0, sl],
                                     start=False, stop=False)
                    nc.tensor.matmul(py, oTlo[0], wo_hi[:, 0, sl],
                                     start=False, stop=False)
                    nc.tensor.matmul(py, oThi[1], wo_hi[:, 1, sl],
                                     start=False, stop=False)
                    nc.tensor.matmul(py, oThi[1], wo_lo[:, 1, sl],
                                     start=False, stop=False)
                    nc.tensor.matmul(py, oTlo[1], wo_hi[:, 1, sl],
                                     start=False, stop=True)
                    nc.vector.tensor_copy(y_sb[:, sl], py)
                nc.sync.dma_start(y_d[qb * P:(qb + 1) * P, :], y_sb)

            # software pipeline: keep phase1 two s-blocks ahead so the PE
            # always has attention matmuls available while DVE/GPSIMD run
            # the norm/rope chain of upcoming blocks
            import os as _os
            LOOKAHEAD = int(_os.environ.get("K_LOOKAHEAD", "2"))
            for si in range(min(LOOKAHEAD, NSB)):
                phase1(si)
            for si in range(NSB):
                attention(si)
                if si + LOOKAHEAD < NSB:
                    phase1(si + LOOKAHEAD)

    nc.finalize()
    return nc


_NC = None


def _get_nc():
    global _NC
    if _NC is None:
        _NC = _build_nc()
    return _NC


def _hilo(a):
    hi = a.astype(ml_dtypes.bfloat16)
    lo = (a - hi.astype(np.float32)).astype(ml_dtypes.bfloat16)
    return hi, lo


def kernel(x, rope_freqs, wq, wk, wv, wo, q_norm_w, k_norm_w, diff_lambda):
    x = np.asarray(x, dtype=np.float32)
    rope_freqs = np.asarray(rope_freqs, dtype=np.float32)
    wq, wk, wv, wo = (np.asarray(a, dtype=np.float32) for a in (wq, wk, wv, wo))
    q_norm_w = np.asarray(q_norm_w, dtype=np.float32)
    k_norm_w = np.asarray(k_norm_w, dtype=np.float32)
    diff_lambda = np.asarray(diff_lambda, dtype=np.float32)

    cos = np.repeat(rope_freqs[:, :, 0], 2, axis=1).astype(np.float32)
    sin = np.repeat(rope_freqs[:, :, 1], 2, axis=1).astype(np.float32)
    sin_s = sin.copy()
    sin_s[:, 0::2] *= -1.0
    wnorm = np.concatenate(
        [np.tile(q_norm_w, 2 * G), np.tile(k_norm_w, 2)]
    ).astype(np.float32)

    in_maps = []
    for c in range(8):
        b, j = divmod(c, KV)
        w_all_t = np.ascontiguousarray(
            np.concatenate(
                [
                    wq[EQ * j:EQ * (j + 1), :],
                    wk[EK * j:EK * (j + 1), :],
                    wv[EV * j:EV * (j + 1), :],
                ],
                axis=0,
            ).T
        )
        wo_t = np.ascontiguousarray(wo[:, 2 * P * j:2 * P * (j + 1)].T)
        w_hi, w_lo = _hilo(w_all_t)
        wo_hi, wo_lo = _hilo(wo_t)
        in_maps.append(
            {
                "x": np.ascontiguousarray(x[b]),
                "w_hi": w_hi,
                "w_lo": w_lo,
                "wo_hi": wo_hi,
                "wo_lo": wo_lo,
                "cos_d": cos,
                "sin_s": sin_s,
                "wnorm": wnorm,
                "lam": diff_lambda.reshape(1),
            }
        )

    nc = _get_nc()
    trace = os.environ.get("KERNEL_TRACE") == "1"
    res = run_bass_kernel_spmd(nc, in_maps, core_ids=list(range(8)), trace=trace)
    if trace and res.exec_time_ns is not None:
        print(f"HW exec time: {res.exec_time_ns} ns")

    out = np.zeros((B, S, D), dtype=np.float32)
    for c in range(8):
        b = c // KV
        out[b] += res.results[c]["y"]
    return out


# revision 28
# speedup vs baseline: 1.0403x; 1.0242x over previous
"""Trainium2 Bass kernel for nn_CausalAttention (diff-attention with QK-norm,
RoPE, GQA, tanh soft-cap, causal softmax).

Sharding: 8 cores = (batch b in {0,1}) x (kv-group j in {0..3}).
Each core handles one batch element and the 4 query heads of one kv head.

Precision/perf strategy (fp32 matmuls cost 4 cyc/row on TRN2):
  - big GEMMs (projections, scores, O-proj) run as 3 bf16 matmuls on an
    exact hi/lo bf16 split of each operand (error ~= dropped lo*lo term,
    ~1.6e-5 relative) at 1 cyc/row each
  - attn_diff^T is built by two accumulated fp32 PE transpose-mode matmuls
    (2 cyc/row) of 1/r1-scaled e1 and (-lambda/r2)-scaled e2
  - AV and everything touching attention weights stays fp32
  - RMSNorm rsqrt runs as Newton iteration on DVE so ScalarE only ever
    needs the exp_and_others table (tanh+exp), avoiding table reloads
"""

import os
import sys

import numpy as np

if "/opt/trn_rl_repo" not in sys.path:
    sys.path.insert(0, "/opt/trn_rl_repo")

import ml_dtypes
import concourse.bass as bass
import concourse.mybir as mybir
import concourse.tile as tile
from concourse import bacc
from concourse.bass_utils import run_bass_kernel_spmd
from concourse.masks import make_identity

B, S, D = 2, 2048, 1024
H, KV, HD = 16, 4, 64
G = H // KV          # q heads per kv head (= heads per core)
CAP = 50.0
EPS = 1e-6
SCALE = 1.0 / 8.0    # 1/sqrt(HD)
P = 128
NSB = S // P         # 16 s-blocks
EQ = G * 2 * HD      # 512 q-projection cols per core
EK = 2 * HD          # 128 k-projection cols per core
EV = HD              # 64  v-projection cols per core
EQK = EQ + EK        # 640 cols needing norm+rope
EALL = EQ + EK + EV  # 704 projection cols per core
NG = EQK // HD       # 10 rmsnorm groups
KT = D // P          # 8 contraction tiles
MASK_FILL = -1.0e4   # exp(50 * -1e4) == 0 in fp32

F32 = mybir.dt.float32
BF16 = mybir.dt.bfloat16
MULT = mybir.AluOpType.mult
ADD = mybir.AluOpType.add
SUB = mybir.AluOpType.subtract


def _build_nc():
    nc = bacc.Bacc()
    x_d = nc.declare_dram_parameter("x", [S, D], F32, isOutput=False)
    whi_d = nc.declare_dram_parameter("w_hi", [D, EALL], BF16, isOutput=False)
    wlo_d = nc.declare_dram_parameter("w_lo", [D, EALL], BF16, isOutput=False)
    wohi_d = nc.declare_dram_parameter("wo_hi", [2 * P, D], BF16, isOutput=False)
    wolo_d = nc.declare_dram_parameter("wo_lo", [2 * P, D], BF16, isOutput=False)
    cos_d = nc.declare_dram_parameter("cos_d", [S, HD], F32, isOutput=False)
    sin_d = nc.declare_dram_parameter("sin_s", [S, HD], F32, isOutput=False)
    wn_d = nc.declare_dram_parameter("wnorm", [EQK], F32, isOutput=False)
    lam_d = nc.declare_dram_parameter("lam", [1], F32, isOutput=False)
    y_d = nc.declare_dram_parameter("y", [S, D], F32, isOutput=True)

    Tanh = mybir.ActivationFunctionType.Tanh
    Exp = mybir.ActivationFunctionType.Exp

    with tile.TileContext(nc) as tc:
        with (
            tc.tile_pool(name="singles", bufs=1) as singles,
            tc.tile_pool(name="persist", bufs=1) as persist,
            tc.tile_pool(name="work", bufs=2) as work,
            tc.tile_pool(name="tpool", bufs=3) as tpool,
            tc.tile_pool(name="atpool", bufs=3) as atpool,
            tc.tile_pool(name="small", bufs=9) as small,
            tc.tile_pool(name="psA", bufs=2, space="PSUM") as psA,
            tc.tile_pool(name="psB", bufs=2, space="PSUM") as psB,
            tc.tile_pool(name="psAT", bufs=2, space="PSUM") as psAT,
        ):
            # ---- one-time setup ----
            w_hi = singles.tile([P, KT, EALL], BF16)
            nc.sync.dma_start(w_hi, whi_d.rearrange("(t p) e -> p t e", p=P))
            w_lo = singles.tile([P, KT, EALL], BF16)
            nc.sync.dma_start(w_lo, wlo_d.rearrange("(t p) e -> p t e", p=P))
            wo_hi = singles.tile([P, 2, D], BF16)
            nc.sync.dma_start(wo_hi, wohi_d.rearrange("(t p) e -> p t e", p=P))
            wo_lo = singles.tile([P, 2, D], BF16)
            nc.sync.dma_start(wo_lo, wolo_d.rearrange("(t p) e -> p t e", p=P))
            cos_sb = singles.tile([P, NSB, HD], F32)
            nc.sync.dma_start(cos_sb, cos_d.rearrange("(n p) f -> p n f", p=P))
            sin_sb = singles.tile([P, NSB, HD], F32)
            nc.sync.dma_start(sin_sb, sin_d.rearrange("(n p) f -> p n f", p=P))

            def part_bcast(handle):
                ap = handle[:]
                return bass.AP(tensor=ap.tensor, offset=ap.offset, ap=[[0, P], *ap.ap])

            wn_sb = singles.tile([P, EQK], F32)
            nc.gpsimd.dma_start(wn_sb, part_bcast(wn_d))
            lam_sb = singles.tile([P, 1], F32)
            nc.gpsimd.dma_start(lam_sb, part_bcast(lam_d))
            ident = singles.tile([P, P], F32)
            make_identity(nc, ident)

            # persistent per-core activation storage
            v_sb = [persist.tile([P, EV], F32, name=f"v{i}", tag=f"v{i}")
                    for i in range(NSB)]
            qThi = [persist.tile([P, G, P], BF16, name=f"qThi{i}", tag=f"qThi{i}")
                    for i in range(NSB)]
            qTlo = [persist.tile([P, G, P], BF16, name=f"qTlo{i}", tag=f"qTlo{i}")
                    for i in range(NSB)]
            kThi = [persist.tile([P, 512], BF16, name=f"kThi{i}", tag=f"kThi{i}")
                    for i in range(NSB // 4)]
            kTlo = [persist.tile([P, 512], BF16, name=f"kTlo{i}", tag=f"kTlo{i}")
                    for i in range(NSB // 4)]

            def bcast_groups(src2d, n):
                return bass.AP(
                    tensor=src2d.tensor,
                    offset=src2d.offset,
                    ap=[src2d.ap[0], [0, n], src2d.ap[-1]],
                )

            def hilo_evict(psrc, hi, lo):
                """psum fp32 -> bf16 hi + bf16 lo (exact split)"""
                nc.vector.tensor_copy(hi, psrc)
                nc.vector.scalar_tensor_tensor(
                    out=lo, in0=hi, scalar=-1.0, in1=psrc, op0=MULT, op1=ADD
                )

            def phase1(si):
                """projections + rmsnorm + rope + transposes for s-block si"""
                x_sb = work.tile([P, D], F32, tag="x")
                nc.scalar.dma_start(x_sb, x_d[si * P:(si + 1) * P, :])
                # transpose x block: [s,d] -> [d,s], split to bf16 hi/lo
                xThi = work.tile([P, KT, P], BF16, tag="xThi")
                xTlo = work.tile([P, KT, P], BF16, tag="xTlo")
                for half in range(2):
                    pt = psB.tile([P, 512], F32, tag="pose")
                    for t in range(4):
                        tt = 4 * half + t
                        nc.tensor.transpose(
                            pt[:, t * P:(t + 1) * P], x_sb[:, tt * P:(tt + 1) * P],
                            ident,
                        )
                    sl = slice(4 * half, 4 * half + 4)
                    hilo_evict(pt, xThi[:, sl, :], xTlo[:, sl, :])
                # projections via hi/lo bf16 3-matmul
                pp = psA.tile([P, 1024], F32, tag="A")
                for t in range(KT):
                    first, last = t == 0, t == KT - 1
                    for co, cw in ((0, 512), (512, 192)):
                        po = pp[:, co:co + cw]
                        wsl = slice(co, co + cw)
                        nc.tensor.matmul(
                            po, xThi[:, t, :], w_hi[:, t, wsl],
                            start=first, stop=False,
                        )
                        nc.tensor.matmul(
                            po, xThi[:, t, :], w_lo[:, t, wsl],
                            start=False, stop=False,
                        )
                        nc.tensor.matmul(
                            po, xTlo[:, t, :], w_hi[:, t, wsl],
                            start=False, stop=last,
                        )
                # v: plain eviction (fp32)
                nc.vector.tensor_copy(v_sb[si], pp[:, EQK:EALL])
                # rmsnorm stats
                qk0 = work.tile([P, EQK], F32, tag="qk0")
                nc.vector.tensor_copy(qk0, pp[:, 0:EQK])
                sq = work.tile([P, EQK], F32, tag="m1")
                nc.gpsimd.tensor_mul(sq, qk0, qk0)
                ssq = small.tile([P, NG], F32, tag="ssq")
                nc.vector.tensor_reduce(
                    ssq, sq.rearrange("p (g d) -> p g d", d=HD),
                    axis=mybir.AxisListType.X, op=ADD,
                )
                # a = mean_sq + eps; rinv = rsqrt(a) via Newton on DVE
                # (keeps ScalarE on the exp/tanh table only)
                aa = small.tile([P, NG], F32, tag="aa")
                nc.vector.tensor_scalar(
                    out=aa, in0=ssq, scalar1=1.0 / HD, scalar2=EPS,
                    op0=MULT, op1=ADD,
                )
                rinv = small.tile([P, NG], F32, tag="rinv")
                nc.vector.reciprocal(rinv, aa)
                nc.vector.tensor_scalar_min(rinv, rinv, 1.0)
                t_n = small.tile([P, NG], F32, tag="t_n")
                for _ in range(5):
                    nc.vector.tensor_mul(t_n, rinv, rinv)
                    nc.vector.tensor_mul(t_n, t_n, aa)
                    nc.vector.tensor_scalar(
                        out=t_n, in0=t_n, scalar1=-0.5, scalar2=1.5,
                        op0=MULT, op1=ADD,
                    )
                    nc.vector.tensor_mul(rinv, rinv, t_n)
                # apply 1/rms and norm weight
                qk = work.tile([P, EQK], F32, tag="qk")
                for g in range(NG):
                    sl = slice(g * HD, (g + 1) * HD)
                    nc.vector.scalar_tensor_tensor(
                        out=qk[:, sl], in0=qk0[:, sl], scalar=rinv[:, g:g + 1],
                        in1=wn_sb[:, sl], op0=MULT, op1=MULT,
                    )
                # rope: out = qk * cos_dup + swap(qk) * sin_sign
                qkv = qk.rearrange("p (n two) -> p n two", two=2)
                xr = work.tile([P, EQK], F32, tag="qk0")
                xrv = xr.rearrange("p (n two) -> p n two", two=2)
                nc.gpsimd.tensor_copy(xrv[:, :, 0:1], qkv[:, :, 1:2])
                nc.gpsimd.tensor_copy(xrv[:, :, 1:2], qkv[:, :, 0:1])
                cosb = bcast_groups(cos_sb[:, si, :], NG)
                sinb = bcast_groups(sin_sb[:, si, :], NG)
                m1 = work.tile([P, EQK], F32, tag="m1")
                nc.gpsimd.tensor_mul(m1, qk, cosb)
                nc.vector.tensor_mul(xr, xr, sinb)
                nc.vector.tensor_add(qk, m1, xr)
                # transpose q heads and k to [dim, s]; split bf16 hi/lo
                pq = psB.tile([P, 512], F32, tag="pose")
                for h in range(G):
                    nc.tensor.transpose(
                        pq[:, h * P:(h + 1) * P], qk[:, h * P:(h + 1) * P], ident
                    )
                hilo_evict(pq, qThi[si], qTlo[si])
                pk = psB.tile([P, 512], F32, tag="pose")
                nc.tensor.transpose(pk[:, 0:P], qk[:, EQ:EQK], ident)
                ksl = slice((si % 4) * P, (si % 4 + 1) * P)
                hilo_evict(pk[:, 0:P], kThi[si // 4][:, ksl], kTlo[si // 4][:, ksl])

            def attention(qb):
                """attention + O-projection for q-block qb (all 4 heads)"""
                nkb = qb + 1
                L = nkb * P
                oThi = [small.tile([P, P], BF16, name=f"oThi{qb}_{hp}", tag=f"oThi{hp}")
                        for hp in range(2)]
                oTlo = [small.tile([P, P], BF16, name=f"oTlo{qb}_{hp}", tag=f"oTlo{hp}")
                        for hp in range(2)]
                for hp in range(2):
                    at_sb = []
                    for hh in range(2):
                        h = 2 * hp + hh
                        t12 = tpool.tile([P, 2, S], F32, tag="t")
                        t1 = t12[:, 0, :]
                        t2 = t12[:, 1, :]
                        # scores: hi/lo 3-matmul, s1/s2 row-paired
                        for kc in range(0, L, 512):
                            w = min(512, L - kc)
                            sc = psA.tile([P, 1024], F32, tag="A")
                            ci = kc // 512
                            for off, qh in ((0, 0), (512, HD)):
                                khi = kThi[ci][qh:qh + HD, 0:w]
                                klo = kTlo[ci][qh:qh + HD, 0:w]
                                qhi = qThi[qb][qh:qh + HD, h, :]
                                qlo = qTlo[qb][qh:qh + HD, h, :]
                                po = sc[:, off:off + w]
                                nc.tensor.matmul(po, qhi, khi, start=True, stop=False)
                                nc.tensor.matmul(po, qhi, klo, start=False, stop=False)
                                nc.tensor.matmul(po, qlo, khi, start=False, stop=True)
                            # one tanh over both score halves
                            nc.scalar.activation(
                                t12[:, :, kc:kc + w],
                                sc.rearrange("p (m c) -> p m c", m=2)[:, :, 0:w],
                                Tanh, scale=SCALE / CAP,
                            )
                        # causal mask on diagonal block (keep where row >= col)
                        for t in (t1, t2):
                            nc.gpsimd.affine_select(
                                out=t[:, qb * P:L], in_=t[:, qb * P:L],
                                compare_op=mybir.AluOpType.is_ge, fill=MASK_FILL,
                                base=0, pattern=[[-1, P]], channel_multiplier=1,
                            )
                        # exp in place with row-sum accumulation
                        r1 = small.tile([P, 1], F32, tag="r")
                        r2 = small.tile([P, 1], F32, tag="r")
                        nc.scalar.activation(
                            t1[:, 0:L], t1[:, 0:L], Exp, scale=CAP, accum_out=r1
                        )
                        nc.scalar.activation(
                            t2[:, 0:L], t2[:, 0:L], Exp, scale=CAP, accum_out=r2
                        )
                        r1i = small.tile([P, 1], F32, tag="r")
                        nc.vector.reciprocal(r1i, r1)
                        r2i = small.tile([P, 1], F32, tag="r")
                        nc.vector.reciprocal(r2i, r2)
                        nr2i = small.tile([P, 1], F32, tag="r")
                        nc.vector.tensor_scalar(
                            out=nr2i, in0=r2i, scalar1=lam_sb[:, 0:1], scalar2=-1.0,
                            op0=MULT, op1=MULT,
                        )
                        # attn_diff^T via two accumulated fp32 transposes;
                        # e1/e2 normalized per 512-chunk right before their
                        # transposes so PE starts ~1 chunk after r1 is ready
                        a_sb = atpool.tile([P, S], F32, tag="at")
                        for kc in range(0, L, 512):
                            w = min(512, L - kc)
                            nc.vector.tensor_scalar_mul(
                                t1[:, kc:kc + w], t1[:, kc:kc + w], r1i[:, 0:1]
                            )
                            nc.vector.tensor_scalar_mul(
                                t2[:, kc:kc + w], t2[:, kc:kc + w], nr2i[:, 0:1]
                            )
                            at4 = psAT.tile([P, 512], F32, tag="atpo")
                            # one bank-clearing start, then per-element
                            # overwrite (e1, has_written unset) / accumulate
                            # (e2, over e1's bits)
                            for kk in range(0, w, P):
                                sl = slice(kc + kk, kc + kk + P)
                                nc.tensor.matmul(
                                    at4[:, kk:kk + P], t1[:, sl], ident,
                                    is_transpose=True, start=(kk == 0), stop=False,
                                )
                            for kk in range(0, w, P):
                                sl = slice(kc + kk, kc + kk + P)
                                nc.tensor.matmul(
                                    at4[:, kk:kk + P], t2[:, sl], ident,
                                    is_transpose=True, start=False,
                                    stop=(kk + P >= w),
                                )
                            nc.vector.tensor_copy(a_sb[:, kc:kc + w], at4[:, 0:w])
                        at_sb.append(a_sb)
                    # AV: head pair via column tiling
                    po = psAT.tile([P, P], F32, tag="atpo", padded_shape=[P, 512])
                    for kb in range(nkb):
                        sl = slice(kb * P, (kb + 1) * P)
                        nc.tensor.matmul(
                            po[0:HD, :], v_sb[kb], at_sb[0][:, sl],
                            start=(kb == 0), stop=(kb == nkb - 1),
                            tile_position=(0, 0),
                        )
                        nc.tensor.matmul(
                            po[HD:P, :], v_sb[kb], at_sb[1][:, sl],
                            start=(kb == 0), stop=(kb == nkb - 1),
                            tile_position=(0, 64),
                        )
                    hilo_evict(po, oThi[hp], oTlo[hp])
                # O-projection (hi/lo bf16 3-matmul)
                y_sb = work.tile([P, D], F32, tag="y")
                for ch in range(2):
                    py = psB.tile([P, 512], F32, tag="pose")
                    sl = slice(ch * 512, (ch + 1) * 512)
                    nc.tensor.matmul(py, oThi[0], wo_hi[:, 0, sl],
                                     start=True, stop=False)
                    nc.tensor.matmul(py, oThi[0], wo_lo[:, 0, sl],
                                     start=False, stop=False)
                    nc.tensor.matmul(py, oTlo[0], wo_hi[:, 0, sl],
                                     start=False, stop=False)
                    nc.tensor.matmul(py, oThi[1], wo_hi[:, 1, sl],
                                     start=False, stop=False)
                    nc.tensor.matmul(py, oThi[1], wo_lo[:, 1, sl],
                                     start=False, stop=False)
                    nc.tensor.matmul(py, oTlo[1], wo_hi[:, 1, sl],
                                     start=False, stop=True)
                    nc.vector.tensor_copy(y_sb[:, sl], py)
                nc.sync.dma_start(y_d[qb * P:(qb + 1) * P, :], y_sb)

            # software pipeline: keep phase1 two s-blocks ahead so the PE
            # always has attention matmuls available while DVE/GPSIMD run
            # the norm/rope chain of upcoming blocks
            import os as _os
            LOOKAHEAD = int(_os.environ.get("K_LOOKAHEAD", "2"))
            for si in range(min(LOOKAHEAD, NSB)):
                phase1(si)
            for si in range(NSB):
                attention(si)
                if si + LOOKAHEAD < NSB:
                    phase1(si + LOOKAHEAD)

    nc.finalize()
    return nc


_NC = None


def _get_nc():
    global _NC
    if _NC is None:
        _NC = _build_nc()
    return _NC


def _hilo(a):
    hi = a.astype(ml_dtypes.bfloat16)
    lo = (a - hi.astype(np.float32)).astype(ml_dtypes.bfloat16)
    return hi, lo


def kernel(x, rope_freqs, wq, wk, wv, wo, q_norm_w, k_norm_w, diff_lambda):
    x = np.asarray(x, dtype=np.float32)
    rope_freqs = np.asarray(rope_freqs, dtype=np.float32)
    wq, wk, wv, wo = (np.asarray(a, dtype=np.float32) for a in (wq, wk, wv, wo))
    q_norm_w = np.asarray(q_norm_w, dtype=np.float32)
    k_norm_w = np.asarray(k_norm_w, dtype=np.float32)
    diff_lambda = np.asarray(diff_lambda, dtype=np.float32)

    cos = np.repeat(rope_freqs[:, :, 0], 2, axis=1).astype(np.float32)
    sin = np.repeat(rope_freqs[:, :, 1], 2, axis=1).astype(np.float32)
    sin_s = sin.copy()
    sin_s[:, 0::2] *= -1.0
    wnorm = np.concatenate(
        [np.tile(q_norm_w, 2 * G), np.tile(k_norm_w, 2)]
    ).astype(np.float32)

    in_maps = []
    for c in range(8):
        b, j = divmod(c, KV)
        w_all_t = np.ascontiguousarray(
            np.concatenate(
                [
                    wq[EQ * j:EQ * (j + 1), :],
                    wk[EK * j:EK * (j + 1), :],
                    wv[EV * j:EV * (j + 1), :],
                ],
                axis=0,
            ).T
        )
        wo_t = np.ascontiguousarray(wo[:, 2 * P * j:2 * P * (j + 1)].T)
        w_hi, w_lo = _hilo(w_all_t)
        wo_hi, wo_lo = _hilo(wo_t)
        in_maps.append(
            {
                "x": np.ascontiguousarray(x[b]),
                "w_hi": w_hi,
                "w_lo": w_lo,
                "wo_hi": wo_hi,
                "wo_lo": wo_lo,
                "cos_d": cos,
                "sin_s": sin_s,
                "wnorm": wnorm,
                "lam": diff_lambda.reshape(1),
            }
        )

    nc = _get_nc()
    trace = os.environ.get("KERNEL_TRACE") == "1"
    res = run_bass_kernel_spmd(nc, in_maps, core_ids=list(range(8)), trace=trace)
    if trace and res.exec_time_ns is not None:
        print(f"HW exec time: {res.exec_time_ns} ns")

    out = np.zeros((B, S, D), dtype=np.float32)
    for c in range(8):
        b = c // KV
        out[b] += res.results[c]["y"]
    return out
